# revision 1
# baseline (speedup 1.0000x reference)
"""Additive attention kernel for Trainium2 (8 NeuronCores, SPMD).

Reference computation (B=4, L=1024, D=256, U=128):
    q = X @ W1                                   [B,L,U]
    k = X @ W2                                   [B,L,U]
    g = tanh(q[:,:,None,:] + k[:,None,:,:] + b1) [B,L,L,U]
    s = sigmoid(g @ W3 + b2)                     [B,L,L]
    out = s @ X                                  [B,L,D]

Sharding: 8 cores = (batch b, query-half h).  Each core handles 512 queries
against all 1024 keys of its batch.

Per-core dataflow (u=128 on SBUF partitions):
    qTb[u,q] = (W1^T X_q^T)[u,q] + b1[u]  fp16   (PE + ACT-identity)
    kT[u,k]  = (W2^T X_b^T)[u,k]          fp32   (PE)
    per key k:  gin[u,:] = qTb[u,:] + kT[u,k]    (DVE tensor_scalar, fp16)
    tanh in big fp16 [128, KB*512] tiles         (ACT - the bottleneck)
    psT[k%128, q] = sum_u W3[u]*gt[u,(k,q)]      (PE fp16, shifted-column Wd)
    scoreT[k,q] = sigmoid(psT + b2)              (ACT)
    out[q,d] += scoreT_kb^T @ X_kb               (PE fp32, accumulated per kb)
"""

import numpy as np

B, L, D, U = 4, 1024, 256, 128
QH = L // 2          # queries per core
KB = 32              # keys per tanh chunk (steady state)
N_CORES = 8

_CACHE = {}
LAST_RESULTS = None


def _build_program():
    import os
    import concourse.bass as bass
    import concourse.bacc as bacc
    import concourse.mybir as mybir
    import concourse.tile as tile
    from concourse import masks

    f32 = mybir.dt.float32
    f16 = mybir.dt.float16
    AF = mybir.ActivationFunctionType

    nc = bacc.Bacc(
        "TRN2",
        target_bir_lowering=False,
        debug=False,
        enable_asserts=False,
        num_devices=N_CORES,
    )

    Xb = nc.dram_tensor("Xb", [L, D], f32, kind="ExternalInput")
    Xq = nc.dram_tensor("Xq", [QH, D], f32, kind="ExternalInput")
    W1 = nc.dram_tensor("W1", [D, U], f32, kind="ExternalInput")
    W2 = nc.dram_tensor("W2", [D, U], f32, kind="ExternalInput")
    W3 = nc.dram_tensor("W3v", [U, 1], f32, kind="ExternalInput")
    b1 = nc.dram_tensor("b1", [U, 1], f32, kind="ExternalInput")
    b2 = nc.dram_tensor("b2", [1, 1], f32, kind="ExternalInput")
    out = nc.dram_tensor("out", [QH, D], f32, kind="ExternalOutput")

    NLB = L // 128   # 8 key blocks
    NQB = QH // 128  # 4 query blocks
    NDB = D // 128   # 2 d blocks

    # chunk sizes per key-block: ramp up in the first block so the first
    # tanh starts as early as possible; ramp down at the very end so the
    # final sigmoid/output dependency chain is short
    WARM = [4, 4, 8, 16, 32, 32, 32]
    COOL = [32, 32, 32, 16, 8, 8]
    assert sum(WARM) == 128 and sum(COOL) == 128

    with tile.TileContext(nc) as tc:
        with (
            tc.tile_pool(name="const", bufs=1) as cp,
            tc.tile_pool(name="ginA", bufs=1) as ginpA,
            tc.tile_pool(name="gtA", bufs=1) as gtpA,
            tc.tile_pool(name="ginB", bufs=1) as ginpB,
            tc.tile_pool(name="gtB", bufs=1) as gtpB,
            tc.tile_pool(name="score", bufs=2) as scp,
            tc.tile_pool(name="outs", bufs=2) as outp,
            tc.tile_pool(name="pre_ps", bufs=2, space="PSUM") as prepsum,
            tc.tile_pool(name="score_ps", bufs=2, space="PSUM") as scorepsum,
            tc.tile_pool(name="out_ps", bufs=1, space="PSUM") as outpsum,
        ):
            ident = cp.tile([128, 128], f32)
            masks.make_identity(nc, ident[:])

            # ---- load inputs (query chain first; spread across the two
            # HWDGE queues: sync + scalar) ----
            Xqs = cp.tile([128, NQB, D], f32)
            nc.sync.dma_start(
                Xqs[:, 0:NQB // 2, :],
                Xq[0:QH // 2].rearrange("(qb p) d -> p qb d", p=128))
            nc.scalar.dma_start(
                Xqs[:, NQB // 2:, :],
                Xq[QH // 2:QH].rearrange("(qb p) d -> p qb d", p=128))
            W1s = cp.tile([128, NDB, U], f32)
            nc.sync.dma_start(W1s[:], W1[:].rearrange("(db p) u -> p db u", p=128))
            b1s = cp.tile([128, 1], f32)
            nc.sync.dma_start(b1s[:], b1[:])
            W3s = cp.tile([128, 1], f32)
            nc.scalar.dma_start(W3s[:], W3[:])
            b2s = cp.tile([1, 1], f32)
            nc.scalar.dma_start(b2s[:], b2[:])
            W2s = cp.tile([128, NDB, U], f32)
            nc.scalar.dma_start(W2s[:], W2[:].rearrange("(db p) u -> p db u", p=128))
            # X_b natural (part = l%128); first key block in its own tile
            # so the early-kT chain doesn't wait for the full transfer
            XsA = cp.tile([128, 1, D], f32)
            nc.sync.dma_start(
                XsA[:], Xb[0:128].rearrange("(kb p) d -> p kb d", p=128))
            XsB = cp.tile([128, NLB // 2 - 1, D], f32)
            nc.sync.dma_start(
                XsB[:],
                Xb[128:L // 2].rearrange("(kb p) d -> p kb d", p=128))
            Xs1 = cp.tile([128, NLB // 2, D], f32)
            nc.scalar.dma_start(
                Xs1[:],
                Xb[L // 2:L].rearrange("(kb p) d -> p kb d", p=128))

            def xs_slice(kb):
                if kb == 0:
                    return XsA[:, 0, :]
                if kb < NLB // 2:
                    return XsB[:, kb - 1, :]
                return Xs1[:, kb - NLB // 2, :]

            ones1 = cp.tile([1, 128], f32)
            nc.vector.memset(ones1[:], 1.0)
            # Wd[:, 128] = W3, zero elsewhere.  Slicing Wd[:, 128-j:256-j]
            # yields a [128,128] stationary operand whose only nonzero
            # column is j - so matmul accumulates W3^T @ g into PSUM row j.
            Wd = cp.tile([128, 2 * 128], f16)
            nc.vector.memset(Wd[:], 0.0)
            nc.vector.tensor_copy(Wd[:, 128:129], W3s[:])

            # ---- transpose X_q then X_b (PE transpose via identity) ----
            XqTs = cp.tile([128, NDB, QH], f32)    # X_q^T
            for qb in range(NQB):
                for db in range(NDB):
                    tp = prepsum.tile([128, 128], f32, tag="pre")
                    nc.tensor.transpose(
                        tp[:], Xqs[:, qb, db * 128:(db + 1) * 128], ident[:]
                    )
                    nc.vector.tensor_copy(
                        XqTs[:, db, qb * 128:(qb + 1) * 128], tp[:]
                    )

            # ---- qTb[u,q] = W1^T Xq^T + b1  (fp16) ----
            qpre = prepsum.tile([128, QH], f32, tag="pre")
            for db in range(NDB):
                nc.tensor.matmul(
                    qpre[:], W1s[:, db, :], XqTs[:, db, :],
                    start=(db == 0), stop=(db == NDB - 1),
                )
            qTb = cp.tile([128, QH], f16)
            nc.scalar.activation(qTb[:], qpre[:], AF.Identity, bias=b1s[:])

            # ---- broadcast b2 across partitions: b2col = ones1.T @ b2s ----
            tpb = prepsum.tile([128, 1], f32, tag="pre")
            nc.tensor.matmul(tpb[:], ones1[:], b2s[:])
            b2col = cp.tile([128, 1], f32)
            nc.vector.tensor_copy(b2col[:], tpb[:])

            # ---- early kT for the first 32 keys (unblocks warmup chunks
            # while the rest of Xb is still being transposed) ----
            XT32 = cp.tile([128, NDB, 128], f32)
            for db in range(NDB):
                tp = prepsum.tile([128, 128], f32, tag="pre")
                nc.tensor.transpose(
                    tp[:], XsA[:, 0, db * 128:(db + 1) * 128], ident[:]
                )
                nc.vector.tensor_copy(XT32[:, db, :], tp[:])
            kpre32 = prepsum.tile([128, 32], f32, tag="pre")
            for db in range(NDB):
                nc.tensor.matmul(
                    kpre32[:], W2s[:, db, :], XT32[:, db, 0:32],
                    start=(db == 0), stop=(db == NDB - 1),
                )
            kT32 = cp.tile([128, 32], f32)
            nc.vector.tensor_copy(kT32[:], kpre32[:])

            # ---- kT[u,k] = W2^T Xb^T, in two [128,512] tiles ----
            kT = []
            for lh in range(2):
                XTl = cp.tile([128, NDB, QH], f32, tag=f"XT{lh}",
                              name=f"XT{lh}")
                for lb in range(NLB // 2):
                    for db in range(NDB):
                        tp = prepsum.tile([128, 128], f32, tag="pre")
                        nc.tensor.transpose(
                            tp[:],
                            xs_slice(lh * (NLB // 2) + lb)[
                                :, db * 128:(db + 1) * 128],
                            ident[:]
                        )
                        nc.vector.tensor_copy(
                            XTl[:, db, lb * 128:(lb + 1) * 128], tp[:]
                        )
                kpre = prepsum.tile([128, QH], f32, tag="pre")
                for db in range(NDB):
                    nc.tensor.matmul(
                        kpre[:], W2s[:, db, :], XTl[:, db, :],
                        start=(db == 0), stop=(db == NDB - 1),
                    )
                kTl = cp.tile([128, QH], f32, tag=f"kT{lh}", name=f"kT{lh}")
                nc.vector.tensor_copy(kTl[:], kpre[:])
                kT.append(kTl)

            # address padding: the DVE<->ACT SBUF bank interaction is
            # sensitive to the absolute placement of the gin/gt pools;
            # this pad restores the empirically good congruence class
            pad = cp.tile([128, int(os.environ.get("PAD_F32", "3808"))], f32)
            nc.gpsimd.memset(pad[:, 0:1], 0.0)

            # ---- main loop over key blocks ----
            # one accumulator tile per query sub-block: each gets its own
            # PSUM bank (matmul start=True zeroes the WHOLE bank)
            po = [outpsum.tile([128, D], f32, tag=f"po{qs}", name=f"po{qs}")
                  for qs in range(NQB)]
            def emit_score(kb, psT):
                # sigmoid + fold this key block into the output accumulators
                scT = scp.tile([128, QH], f32, tag="scT", name="scT")
                nc.scalar.activation(scT[:], psT[:], AF.Sigmoid, bias=b2col[:])
                for qs in range(NQB):
                    nc.tensor.matmul(
                        po[qs][:], scT[:, qs * 128:(qs + 1) * 128],
                        xs_slice(kb),
                        start=(kb == 0), stop=(kb == NLB - 1),
                        skip_group_check=True,
                    )

            ci = 0  # global chunk counter for A/B buffer alternation
            pending = None  # (kb, psT) whose sigmoid is deferred one chunk
            for kb in range(NLB):
                psT = scorepsum.tile([128, QH], f32)
                if kb == 0:
                    sizes = WARM
                elif kb == NLB - 1:
                    sizes = COOL
                else:
                    sizes = [KB] * (128 // KB)
                kloc = 0
                for sz in sizes:
                    ginp = ginpA if ci % 2 == 0 else ginpB
                    gtp = gtpA if ci % 2 == 0 else gtpB
                    ci += 1
                    gin = ginp.tile([128, KB * QH], f16, tag="gin",
                                    name=f"gin{ci % 2}")
                    for j in range(sz):
                        k = kb * 128 + kloc + j
                        if k < 32:
                            kcol = kT32[:, k:k + 1]
                        else:
                            kcol = kT[k // QH][:, (k % QH):(k % QH) + 1]
                        nc.vector.tensor_scalar_add(
                            gin[:, j * QH:(j + 1) * QH], qTb[:], kcol,
                        )
                    gt = gtp.tile([128, KB * QH], f16, tag="gt",
                                  name=f"gt{ci % 2}")
                    nc.scalar.activation(
                        gt[:, :sz * QH], gin[:, :sz * QH], AF.Tanh
                    )
                    # previous block's sigmoid goes on the ACT queue right
                    # after this tanh, so ACT never stalls waiting for PE
                    if pending is not None:
                        emit_score(*pending)
                        pending = None
                    for j in range(sz):
                        nc.tensor.matmul(
                            psT[:], Wd[:, 128 - (kloc + j):256 - (kloc + j)],
                            gt[:, j * QH:(j + 1) * QH],
                            start=(kloc + j == 0), stop=(kloc + j == 127),
                        )
                    kloc += sz
                pending = (kb, psT)
            emit_score(*pending)

            # ---- write out (spread across both DMA queues) ----
            for qs in range(NQB):
                ot = outp.tile([128, D], f32, tag="ot", name="ot")
                nc.vector.tensor_copy(ot[:], po[qs][:])
                eng = nc.sync if qs % 2 == 0 else nc.scalar
                eng.dma_start(out[qs * 128:(qs + 1) * 128, :], ot[:])

    nc.compile()
    return nc


def _get_nc():
    if "nc" not in _CACHE:
        _CACHE["nc"] = _build_program()
    return _CACHE["nc"]


def kernel(X, W1, W2, W3, bias1, bias2, trace=False):
    global LAST_RESULTS
    from concourse.bass_utils import run_bass_kernel_spmd

    X = np.ascontiguousarray(np.asarray(X, dtype=np.float32))
    W1 = np.ascontiguousarray(np.asarray(W1, dtype=np.float32))
    W2 = np.ascontiguousarray(np.asarray(W2, dtype=np.float32))
    W3 = np.ascontiguousarray(np.asarray(W3, dtype=np.float32))
    b1 = np.ascontiguousarray(np.asarray(bias1, dtype=np.float32).reshape(U, 1))
    b2 = np.ascontiguousarray(np.asarray(bias2, dtype=np.float32).reshape(1, 1))

    nc = _get_nc()
    in_maps = []
    for c in range(N_CORES):
        b, h = c // 2, c % 2
        in_maps.append({
            "Xb": X[b],
            "Xq": np.ascontiguousarray(X[b, h * QH:(h + 1) * QH]),
            "W1": W1,
            "W2": W2,
            "W3v": W3,
            "b1": b1,
            "b2": b2,
        })

    res = run_bass_kernel_spmd(nc, in_maps, core_ids=list(range(N_CORES)),
                               trace=trace)
    LAST_RESULTS = res

    out = np.empty((B, L, D), dtype=np.float32)
    for c in range(N_CORES):
        b, h = c // 2, c % 2
        out[b, h * QH:(h + 1) * QH] = res.results[c]["out"]
    return out



# revision 3
# speedup vs baseline: 2.6083x; 2.6083x over previous
"""Additive attention kernel for Trainium2 (8 NeuronCores, SPMD).

Reference computation (B=4, L=1024, D=256, U=128):
    q = X @ W1                                   [B,L,U]
    k = X @ W2                                   [B,L,U]
    g = tanh(q[:,:,None,:] + k[:,None,:,:] + b1) [B,L,L,U]
    s = sigmoid(g @ W3 + b2)                     [B,L,L]
    out = s @ X                                  [B,L,D]

Sharding: 8 cores = (batch b, query-half h).  Each core handles 512 queries
against all 1024 keys of its batch.

Algorithm: the L*L*U tanh tensor is never materialized.  tanh is
approximated by a 4-term Fourier sine series  tanh(x) ~ sum_r c_r sin(w_r x)
(fit on |x|<=13, weighted rms ~1e-2; end-to-end rel err ~3.5e-3), and
sin(w(q+k)) = sin(wq)cos(wk) + cos(wq)sin(wk) turns the score computation
into a plain matmul with contraction dim 2R*U = 8*128:

    z[q,k] = sum_u W3_u tanh(q_u + k_u + b1_u)
           ~ sum_i Fq_i[u,q] . Fk_i[u,k]        (8 accumulating PE matmuls)

The HW Sin spline is only valid on [-pi, pi], so arguments are range-reduced
with the fp32 magic-constant trick (f = u - round(u), round via +-1.5*2^23)
on DVE (k side) and GPSIMD (q side); cos comes from a +0.25-shifted copy of
the reduction.  The lowest frequency needs no reduction.

sigmoid(z+b2) = 0.5*tanh(0.5*z + 0.5*b2) + 0.5 keeps everything in one ACT
table set (sin+tanh).  The 0.5* factors are folded into Xh = X/2 (with
W*2 compensating in the q/k projections) and the +0.5 term becomes a rank-1
colsum(Xh) matmul accumulated into the output PSUM group.
"""

import numpy as np

B, L, D, U = 4, 1024, 256, 128
QH = L // 2          # queries per core
N_CORES = 8
R = 4

# Fourier fit of tanh on [-13,13], gaussian-weighted (sigma^2=2.67)
OMEGA = [0.2177, 0.7144, 1.48488, 2.49552]
COEF = [1.226339, 0.458469, 0.174421, 0.045835]
MAGIC = float(1.5 * 2 ** 23)
TWO_PI = float(2 * np.pi)

_CACHE = {}
LAST_RESULTS = None


def _build_program():
    import concourse.bass as bass
    import concourse.bacc as bacc
    import concourse.mybir as mybir
    import concourse.tile as tile
    from concourse import masks

    f32 = mybir.dt.float32
    f16 = mybir.dt.float16
    AF = mybir.ActivationFunctionType
    ALU = mybir.AluOpType

    nc = bacc.Bacc(
        "TRN2",
        target_bir_lowering=False,
        debug=False,
        enable_asserts=False,
        num_devices=N_CORES,
    )

    Xb = nc.dram_tensor("Xb", [L, D], f32, kind="ExternalInput")
    Xq = nc.dram_tensor("Xq", [QH, D], f32, kind="ExternalInput")
    W1 = nc.dram_tensor("W1", [D, U], f32, kind="ExternalInput")
    W2 = nc.dram_tensor("W2", [D, U], f32, kind="ExternalInput")
    W3 = nc.dram_tensor("W3v", [U, 1], f32, kind="ExternalInput")
    b1 = nc.dram_tensor("b1", [U, 1], f32, kind="ExternalInput")
    b2 = nc.dram_tensor("b2", [1, 1], f32, kind="ExternalInput")
    out = nc.dram_tensor("out", [QH, D], f32, kind="ExternalOutput")

    NLB = L // 128   # 8 key blocks
    NQB = QH // 128  # 4 query blocks
    NDB = D // 128   # 2 d blocks
    NI = 2 * R       # factor pairs

    with tile.TileContext(nc) as tc:
        with (
            tc.tile_pool(name="const", bufs=1) as cp,
            tc.tile_pool(name="score", bufs=2) as scp,
            tc.tile_pool(name="outs", bufs=2) as outp,
            tc.tile_pool(name="pre_ps", bufs=2, space="PSUM") as prepsum,
            tc.tile_pool(name="score_ps", bufs=2, space="PSUM") as scorepsum,
            tc.tile_pool(name="out_ps", bufs=1, space="PSUM") as outpsum,
        ):
            # ---- input DMA, split across the two HWDGE queues ----
            Xqs = cp.tile([128, NQB, D], f32)
            nc.sync.dma_start(
                Xqs[:, 0:NQB // 2, :],
                Xq[0:QH // 2].rearrange("(qb p) d -> p qb d", p=128))
            nc.scalar.dma_start(
                Xqs[:, NQB // 2:, :],
                Xq[QH // 2:QH].rearrange("(qb p) d -> p qb d", p=128))
            W1s = cp.tile([128, NDB, U], f32)
            nc.sync.dma_start(W1s[:], W1[:].rearrange("(db p) u -> p db u", p=128))
            b1s = cp.tile([128, 1], f32)
            nc.sync.dma_start(b1s[:], b1[:])
            W2s = cp.tile([128, NDB, U], f32)
            nc.scalar.dma_start(W2s[:], W2[:].rearrange("(db p) u -> p db u", p=128))
            W3s = cp.tile([128, 1], f32)
            nc.scalar.dma_start(W3s[:], W3[:])
            b2s = cp.tile([1, 1], f32)
            nc.scalar.dma_start(b2s[:], b2[:])
            Xbs = cp.tile([128, NLB, D], f32)
            nc.sync.dma_start(
                Xbs[:, 0:NLB // 2, :],
                Xb[0:L // 2].rearrange("(kb p) d -> p kb d", p=128))
            nc.scalar.dma_start(
                Xbs[:, NLB // 2:, :],
                Xb[L // 2:L].rearrange("(kb p) d -> p kb d", p=128))

            # ---- constants ----
            ident = cp.tile([128, 128], f16)
            masks.make_identity(nc, ident[:])
            halfpi = cp.tile([128, 1], f32)
            nc.vector.memset(halfpi[:], float(np.pi / 2))
            ones_col = cp.tile([128, 1], f16)
            nc.vector.memset(ones_col[:], 1.0)
            ones_row = cp.tile([1, 128], f16)
            nc.vector.memset(ones_row[:], 1.0)
            ones_row32 = cp.tile([1, 128], f32)
            nc.vector.memset(ones_row32[:], 1.0)
            dum = cp.tile([1, 1], f32)
            nc.vector.memset(dum[:], 0.0)
            # early dummy activations to trigger the act-table load at t=0
            dumo = cp.tile([1, 2], f32)
            nc.scalar.activation(dumo[:, 0:1], dum[:], AF.Sin, bias=0.0)
            nc.scalar.activation(dumo[:, 1:2], dum[:], AF.Tanh, bias=0.0)

            # A_r[u] = c_r * W3[u]  (one per frequency)
            Acoef = cp.tile([128, R], f32)
            for r in range(R):
                nc.vector.tensor_scalar_mul(Acoef[:, r:r + 1], W3s[:], COEF[r])

            # ---- fp16 halves/doubles: Xh = X/2, Wh = 2W ----
            Xqh = cp.tile([128, NQB, D], f16)
            nc.gpsimd.tensor_scalar_mul(Xqh[:], Xqs[:], 0.5)
            Xbh = cp.tile([128, NLB, D], f16)
            nc.gpsimd.tensor_scalar_mul(Xbh[:, 0:NLB // 2, :],
                                        Xbs[:, 0:NLB // 2, :], 0.5)
            nc.gpsimd.tensor_scalar_mul(Xbh[:, NLB // 2:, :],
                                        Xbs[:, NLB // 2:, :], 0.5)
            W1h = cp.tile([128, NDB, U], f16)
            nc.vector.tensor_scalar_mul(W1h[:], W1s[:], 2.0)
            W2h = cp.tile([128, NDB, U], f16)
            nc.vector.tensor_scalar_mul(W2h[:], W2s[:], 2.0)

            # ---- transposes (PE, fp16) ----
            xqT = cp.tile([128, NDB, QH], f16)
            for qb in range(NQB):
                for db in range(NDB):
                    tp = prepsum.tile([128, 128], f16, tag="pre", name="tp")
                    nc.tensor.transpose(
                        tp[:], Xqh[:, qb, db * 128:(db + 1) * 128], ident[:])
                    nc.vector.tensor_copy(
                        xqT[:, db, qb * 128:(qb + 1) * 128], tp[:])

            # ---- qT[u,q] = (2W1)^T (Xq/2)^T ----
            qpre = prepsum.tile([128, QH], f32, tag="pre", name="qpre")
            for db in range(NDB):
                nc.tensor.matmul(qpre[:], W1h[:, db, :], xqT[:, db, :],
                                 start=(db == 0), stop=(db == NDB - 1))
            qT = cp.tile([128, QH], f32)
            nc.vector.tensor_copy(qT[:], qpre[:])

            # ---- q-side range reduction (GPSIMD) ----
            # uq[:, j, 0, :] = (w_{j+1}/2pi) qT ; [:, j, 1, :] = +0.25 (cos)
            uq = cp.tile([128, R - 1, 2, QH], f32)
            for j in range(R - 1):
                nc.gpsimd.tensor_scalar_mul(uq[:, j, 0, :], qT[:],
                                            OMEGA[j + 1] / TWO_PI)
            nc.gpsimd.tensor_scalar_add(uq[:, :, 1, :], uq[:, :, 0, :], 0.25)
            ruq = cp.tile([128, R - 1, 2, QH], f32)
            nc.gpsimd.tensor_scalar(ruq[:], uq[:], MAGIC, MAGIC,
                                    op0=ALU.add, op1=ALU.subtract)
            fq = cp.tile([128, R - 1, 2, QH], f32)
            nc.gpsimd.tensor_tensor(fq[:], uq[:], ruq[:], op=ALU.subtract)

            # ---- q factors: Fq[:, 2r+phi, :], phi: 0=sin 1=cos ----
            Fq = cp.tile([128, NI, QH], f16)
            nc.scalar.activation(Fq[:, 0, :], qT[:], AF.Sin,
                                 bias=0.0, scale=OMEGA[0])
            nc.scalar.activation(Fq[:, 1, :], qT[:], AF.Sin,
                                 bias=halfpi[:], scale=OMEGA[0])
            nc.scalar.activation(Fq[:, 2:NI, :], fq[:], AF.Sin,
                                 bias=0.0, scale=TWO_PI)
            # fold A_r = c_r*W3 into the q side
            for i in range(NI):
                nc.vector.tensor_scalar(Fq[:, i, :], Fq[:, i, :],
                                        Acoef[:, i // 2:i // 2 + 1], None,
                                        op0=ALU.mult)

            # ---- Xb transposes + kT = (2W2)^T (Xb/2)^T + b1 ----
            xbT = cp.tile([128, NDB, L], f16)
            for kb in range(NLB):
                for db in range(NDB):
                    tp = prepsum.tile([128, 128], f16, tag="pre", name="tp")
                    nc.tensor.transpose(
                        tp[:], Xbh[:, kb, db * 128:(db + 1) * 128], ident[:])
                    nc.vector.tensor_copy(
                        xbT[:, db, kb * 128:(kb + 1) * 128], tp[:])
            kT = cp.tile([128, L], f32)
            for lh in range(2):
                kpre = prepsum.tile([128, QH], f32, tag="pre", name="kpre")
                for db in range(NDB):
                    nc.tensor.matmul(
                        kpre[:], W2h[:, db, :],
                        xbT[:, db, lh * QH:(lh + 1) * QH],
                        start=(db == 0), stop=(db == NDB - 1))
                nc.vector.tensor_scalar_add(kT[:, lh * QH:(lh + 1) * QH],
                                            kpre[:], b1s[:])

            # ---- k factors (phi FLIPPED: 0=cos 1=sin) + reductions (DVE),
            # emitted per key-half so PE can start scores early ----
            Fk = cp.tile([128, NI, L], f16)
            uk = cp.tile([128, R - 1, 2, L], f32)
            ruk = cp.tile([128, R - 1, 2, L], f32)
            fk = cp.tile([128, R - 1, 2, L], f32)
            for lh in range(2):
                s = slice(lh * QH, (lh + 1) * QH)
                for j in range(R - 1):
                    nc.vector.tensor_scalar_mul(uk[:, j, 1, s], kT[:, s],
                                                OMEGA[j + 1] / TWO_PI)
                nc.vector.tensor_scalar_add(uk[:, :, 0, s], uk[:, :, 1, s],
                                            0.25)
                nc.vector.tensor_scalar(ruk[:, :, :, s], uk[:, :, :, s],
                                        MAGIC, MAGIC,
                                        op0=ALU.add, op1=ALU.subtract)
                nc.vector.tensor_tensor(fk[:, :, :, s], uk[:, :, :, s],
                                        ruk[:, :, :, s], op=ALU.subtract)
                nc.scalar.activation(Fk[:, 0, s], kT[:, s], AF.Sin,
                                     bias=halfpi[:], scale=OMEGA[0])
                nc.scalar.activation(Fk[:, 1, s], kT[:, s], AF.Sin,
                                     bias=0.0, scale=OMEGA[0])
                nc.scalar.activation(Fk[:, 2:NI, s], fk[:, :, :, s], AF.Sin,
                                     bias=0.0, scale=TWO_PI)

            # ---- colsum(Xh) and 0.5*b2 broadcast ----
            csp = prepsum.tile([1, D], f32, tag="pre", name="csp")
            for kb in range(NLB):
                nc.tensor.matmul(csp[:], ones_col[:], Xbh[:, kb, :],
                                 start=(kb == 0), stop=(kb == NLB - 1))
            csh = cp.tile([1, D], f16)
            nc.vector.tensor_copy(csh[:], csp[:])
            tpb = prepsum.tile([128, 1], f32, tag="pre", name="tpb")
            nc.tensor.matmul(tpb[:], ones_row32[:], b2s[:])
            b2h = cp.tile([128, 1], f32)
            nc.vector.tensor_scalar_mul(b2h[:], tpb[:], 0.5)

            # ---- output accumulators; rank-1 colsum term starts the group ----
            po = [outpsum.tile([128, D], f32, tag=f"po{qs}", name=f"po{qs}")
                  for qs in range(NQB)]
            for qs in range(NQB):
                nc.tensor.matmul(po[qs][:], ones_row[:], csh[:],
                                 start=True, stop=False, skip_group_check=True)

            # ---- main loop over key blocks ----
            for kb in range(NLB):
                scpre = scorepsum.tile([128, QH], f32, name="scpre")
                for i in range(NI):
                    nc.tensor.matmul(
                        scpre[:], Fk[:, i, kb * 128:(kb + 1) * 128], Fq[:, i, :],
                        start=(i == 0), stop=(i == NI - 1))
                scT = scp.tile([128, QH], f16, tag="scT", name="scT")
                nc.scalar.activation(scT[:], scpre[:], AF.Tanh,
                                     bias=b2h[:], scale=0.5)
                for qs in range(NQB):
                    nc.tensor.matmul(
                        po[qs][:], scT[:, qs * 128:(qs + 1) * 128],
                        Xbh[:, kb, :],
                        start=False, stop=(kb == NLB - 1),
                        skip_group_check=True)

            # ---- write out ----
            for qs in range(NQB):
                ot = outp.tile([128, D], f32, tag="ot", name="ot")
                nc.vector.tensor_copy(ot[:], po[qs][:])
                eng = nc.sync if qs % 2 == 0 else nc.scalar
                eng.dma_start(out[qs * 128:(qs + 1) * 128, :], ot[:])

    # The act-table chooser picks the first set containing each function,
    # which ping-pongs between exp_and_others (tanh) and trig_and_small
    # (sin) -- 7 reloads at ~2.7us each.  silu_and_others genuinely
    # contains both sin and tanh; restrict membership (indices unchanged,
    # so emitted set ids stay valid) so one load covers the whole kernel.
    from concourse.hw_specs import get_activation_tables
    tabs = get_activation_tables(nc.m.arch)
    for name, fns in tabs.items():
        if name != "silu_and_others":
            fns.discard(AF.Sin)
            fns.discard(AF.Tanh)

    nc.compile()
    return nc


def _get_nc():
    if "nc" not in _CACHE:
        _CACHE["nc"] = _build_program()
    return _CACHE["nc"]


def kernel(X, W1, W2, W3, bias1, bias2, trace=False):
    global LAST_RESULTS
    from concourse.bass_utils import run_bass_kernel_spmd

    X = np.ascontiguousarray(np.asarray(X, dtype=np.float32))
    W1 = np.ascontiguousarray(np.asarray(W1, dtype=np.float32))
    W2 = np.ascontiguousarray(np.asarray(W2, dtype=np.float32))
    W3 = np.ascontiguousarray(np.asarray(W3, dtype=np.float32))
    b1 = np.ascontiguousarray(np.asarray(bias1, dtype=np.float32).reshape(U, 1))
    b2 = np.ascontiguousarray(np.asarray(bias2, dtype=np.float32).reshape(1, 1))

    nc = _get_nc()
    in_maps = []
    for c in range(N_CORES):
        b, h = c // 2, c % 2
        in_maps.append({
            "Xb": X[b],
            "Xq": np.ascontiguousarray(X[b, h * QH:(h + 1) * QH]),
            "W1": W1,
            "W2": W2,
            "W3v": W3,
            "b1": b1,
            "b2": b2,
        })

    res = run_bass_kernel_spmd(nc, in_maps, core_ids=list(range(N_CORES)),
                               trace=trace)
    LAST_RESULTS = res

    out = np.empty((B, L, D), dtype=np.float32)
    for c in range(N_CORES):
        b, h = c // 2, c % 2
        out[b, h * QH:(h + 1) * QH] = res.results[c]["out"]
    return out


# revision 8
# speedup vs baseline: 6.5847x; 2.5246x over previous
"""Additive attention kernel for Trainium2 (8 NeuronCores, SPMD).

Reference computation (B=4, L=1024, D=256, U=128):
    q = X @ W1                                   [B,L,U]
    k = X @ W2                                   [B,L,U]
    g = tanh(q[:,:,None,:] + k[:,None,:,:] + b1) [B,L,L,U]
    s = sigmoid(g @ W3 + b2)                     [B,L,L]
    out = s @ X                                  [B,L,D]

Sharding: 8 cores = (batch b, query-half h).  Each core handles 512 queries
against all 1024 keys of its batch.

Algorithm: the L*L*U tanh tensor is never materialized.  tanh is
approximated by a 4-term Fourier sine series  tanh(x) ~ sum_r c_r sin(w_r x)
(fit on |x|<=13, gaussian-weighted; end-to-end rel err ~3.5e-3), and
sin(w(q+k)) = sin(wq)cos(wk) + cos(wq)sin(wk) turns the score computation
into a plain matmul with contraction dim 2R*U = 8*128:

    z[q,k] = sum_u W3_u tanh(q_u + k_u + b1_u)
           ~ sum_i Fq_i[u,q] . Fk_i[u,k]        (8 accumulating PE matmuls)

The HW Sin spline is only valid on [-pi, pi]; arguments for the three
higher frequencies are range-reduced with the fp32 magic-constant trick
(f = u - round(u), round via +-1.5*2^23) on DVE; cos comes from a
+0.25-shifted copy of the reduction.  The lowest frequency is in range
directly (cos via +pi/2 bias).

sigmoid(z+b2) = 0.5*tanh(0.5*z + 0.5*b2) + 0.5 keeps everything in one ACT
table set (sin+tanh, silu_and_others).  The +0.5 term becomes a rank-1
colsum(X) matmul accumulated into the output PSUM group and the global 0.5
factor is applied in the final PSUM->SBUF output copies.

Matmuls: scores in fp16 (factors written fp16 by ACT directly), all
X-consuming matmuls in float32r (1 cycle/row at free-dim >= 256) so X is
never converted.  PE order is software-pipelined (scores of block kb+1
issue before the outputs of block kb, so the PE never waits on tanh).
"""

import numpy as np

B, L, D, U = 4, 1024, 256, 128
QH = L // 2          # queries per core
N_CORES = 8
R = 4

# Fourier fit of tanh on [-13,13], gaussian-weighted (sigma^2=2.67)
OMEGA = [0.2177, 0.7144, 1.48488, 2.49552]
COEF = [1.226339, 0.458469, 0.174421, 0.045835]
MAGIC = float(1.5 * 2 ** 23)
TWO_PI = float(2 * np.pi)

_CACHE = {}
LAST_RESULTS = None


def _build_program():
    import concourse.bass as bass
    import concourse.bacc as bacc
    import concourse.mybir as mybir
    import concourse.tile as tile
    from concourse import masks

    f32 = mybir.dt.float32
    f32r = mybir.dt.float32r
    f16 = mybir.dt.float16
    AF = mybir.ActivationFunctionType
    ALU = mybir.AluOpType

    nc = bacc.Bacc(
        "TRN2",
        target_bir_lowering=False,
        debug=False,
        enable_asserts=False,
        num_devices=N_CORES,
    )

    Xb = nc.dram_tensor("Xb", [L, D], f32, kind="ExternalInput")
    Xq = nc.dram_tensor("Xq", [QH, D], f32, kind="ExternalInput")
    W1 = nc.dram_tensor("W1", [D, U], f32, kind="ExternalInput")
    W2 = nc.dram_tensor("W2", [D, U], f32, kind="ExternalInput")
    W3 = nc.dram_tensor("W3v", [U, 1], f32, kind="ExternalInput")
    b1 = nc.dram_tensor("b1", [U, 1], f32, kind="ExternalInput")
    b2 = nc.dram_tensor("b2", [1, 1], f32, kind="ExternalInput")
    out = nc.dram_tensor("out", [QH, D], f32, kind="ExternalOutput")

    NLB = L // 128   # 8 key blocks
    NQB = QH // 128  # 4 query blocks
    NDB = D // 128   # 2 d blocks
    NI = 2 * R       # factor pairs

    def rr(ap):
        return ap.bitcast(f32r)

    with tile.TileContext(nc) as tc:
        with (
            tc.tile_pool(name="const", bufs=1) as cp,
            tc.tile_pool(name="score", bufs=2) as scp,
            tc.tile_pool(name="outs", bufs=2) as outp,
            tc.tile_pool(name="pre_ps", bufs=2, space="PSUM") as prepsum,
            tc.tile_pool(name="score_ps", bufs=2, space="PSUM") as scorepsum,
            tc.tile_pool(name="out_ps", bufs=1, space="PSUM") as outpsum,
        ):
            # ---- input DMA across four HWDGE queues ----
            Xqs = cp.tile([128, NQB, D], f32)
            nc.sync.dma_start(
                Xqs[:, 0:NQB // 2, :],
                Xq[0:QH // 2].rearrange("(qb p) d -> p qb d", p=128))
            nc.scalar.dma_start(
                Xqs[:, NQB // 2:, :],
                Xq[QH // 2:QH].rearrange("(qb p) d -> p qb d", p=128))
            W1s = cp.tile([128, NDB, U], f32)
            nc.sync.dma_start(W1s[:], W1[:].rearrange("(db p) u -> p db u", p=128))
            b1s = cp.tile([128, 1], f32)
            nc.sync.dma_start(b1s[:], b1[:])
            W2s = cp.tile([128, NDB, U], f32)
            nc.scalar.dma_start(W2s[:], W2[:].rearrange("(db p) u -> p db u", p=128))
            W3s = cp.tile([128, 1], f32)
            nc.scalar.dma_start(W3s[:], W3[:])
            b2s = cp.tile([1, 1], f32)
            nc.scalar.dma_start(b2s[:], b2[:])
            Xbs = cp.tile([128, NLB, D], f32)
            nc.gpsimd.dma_start(
                Xbs[:, 0:NLB // 2, :],
                Xb[0:L // 2].rearrange("(kb p) d -> p kb d", p=128))
            nc.scalar.dma_start(
                Xbs[:, NLB // 2:NLB * 3 // 4, :],
                Xb[L // 2:L * 3 // 4].rearrange("(kb p) d -> p kb d", p=128))
            nc.sync.dma_start(
                Xbs[:, NLB * 3 // 4:, :],
                Xb[L * 3 // 4:L].rearrange("(kb p) d -> p kb d", p=128))

            # ---- constants ----
            ident = cp.tile([128, 128], f32)
            masks.make_identity(nc, ident[:])
            halfpi = cp.tile([128, 1], f32)
            nc.vector.memset(halfpi[:], float(np.pi / 2))
            ones_col = cp.tile([128, 1], f32)
            nc.vector.memset(ones_col[:], 1.0)
            ones_row = cp.tile([1, 128], f32)
            nc.vector.memset(ones_row[:], 1.0)
            dum = cp.tile([1, 1], f32)
            nc.vector.memset(dum[:], 0.0)
            # early dummy activations to trigger the act-table load at t=0
            dumo = cp.tile([1, 2], f32)
            nc.scalar.activation(dumo[:, 0:1], dum[:], AF.Sin, bias=0.0)
            nc.scalar.activation(dumo[:, 1:2], dum[:], AF.Tanh, bias=0.0)

            # bias tiles: A_r = c_r*W3; k-side r0 sin/cos biases (fold b1);
            # per-frequency (w_r/2pi)*b1 offsets for the reductions
            Acoef = cp.tile([128, R], f32)
            for r in range(R):
                nc.vector.tensor_scalar_mul(Acoef[:, r:r + 1], W3s[:], COEF[r])
            kb_s = cp.tile([128, 1], f32)
            nc.vector.tensor_scalar_mul(kb_s[:], b1s[:], OMEGA[0])
            kb_c = cp.tile([128, 1], f32)
            nc.vector.tensor_scalar(kb_c[:], b1s[:], OMEGA[0],
                                    float(np.pi / 2),
                                    op0=ALU.mult, op1=ALU.add)
            b1u = cp.tile([128, R - 1], f32)
            for j in range(R - 1):
                nc.vector.tensor_scalar_mul(b1u[:, j:j + 1], b1s[:],
                                            OMEGA[j + 1] / TWO_PI)

            # rounded f32r copies of the weights and ones vectors
            W1r = cp.tile([128, NDB, U], f32r)
            nc.vector.tensor_copy(W1r[:], W1s[:])
            W2r = cp.tile([128, NDB, U], f32r)
            nc.vector.tensor_copy(W2r[:], W2s[:])
            ones_col_r = cp.tile([128, 1], f32r)
            nc.vector.tensor_copy(ones_col_r[:], ones_col[:])
            ones_row_r = cp.tile([1, 128], f32r)
            nc.vector.tensor_copy(ones_row_r[:], ones_row[:])

            # ---- Xq transposes (PE fp32 + DVE psum->sbuf copies) ----
            xqT = cp.tile([128, NDB, QH], f32r)
            for qb in range(NQB):
                for db in range(NDB):
                    tp = prepsum.tile([128, 128], f32, tag="pre", name="tp")
                    nc.tensor.transpose(
                        tp[:], Xqs[:, qb, db * 128:(db + 1) * 128], ident[:])
                    nc.vector.tensor_copy(
                        xqT[:, db, qb * 128:(qb + 1) * 128], tp[:])

            # ---- qT[u,q] = W1^T Xq^T (stays in PSUM; ACT/DVE read it) ----
            qpre = prepsum.tile([128, QH], f32, tag="qkpre", name="qpre",
                                bufs=2)
            for db in range(NDB):
                nc.tensor.matmul(qpre[:], W1r[:, db, :], xqT[:, db, :],
                                 start=(db == 0), stop=(db == NDB - 1))

            # ---- q-side factors ----
            Fq = cp.tile([128, NI, QH], f16)
            nc.scalar.activation(Fq[:, 0, :], qpre[:], AF.Sin,
                                 bias=0.0, scale=OMEGA[0])
            nc.scalar.activation(Fq[:, 1, :], qpre[:], AF.Sin,
                                 bias=halfpi[:], scale=OMEGA[0])
            # reductions: slot 0 = sin (u), slot 1 = cos (u + 0.25)
            uq = cp.tile([128, R - 1, 2, QH], f32)
            for j in range(R - 1):
                nc.vector.tensor_scalar_mul(uq[:, j, 0, :], qpre[:],
                                            OMEGA[j + 1] / TWO_PI)
            nc.vector.tensor_scalar_add(uq[:, :, 1, :], uq[:, :, 0, :], 0.25)

            # ---- Xb transposes ----
            xbT = cp.tile([128, NDB, L], f32r)
            for kb in range(NLB):
                for db in range(NDB):
                    tp = prepsum.tile([128, 128], f32, tag="pre", name="tp")
                    nc.tensor.transpose(
                        tp[:], Xbs[:, kb, db * 128:(db + 1) * 128], ident[:])
                    nc.vector.tensor_copy(
                        xbT[:, db, kb * 128:(kb + 1) * 128], tp[:])

            # finish q reduction on DVE, then the merged q sine on ACT
            ruq = cp.tile([128, R - 1, 2, QH], f32)
            nc.vector.tensor_scalar(ruq[:], uq[:], MAGIC, MAGIC,
                                    op0=ALU.add, op1=ALU.subtract)
            fq = cp.tile([128, R - 1, 2, QH], f32)
            nc.vector.tensor_tensor(fq[:], uq[:], ruq[:], op=ALU.subtract)
            nc.scalar.activation(Fq[:, 2:NI, :], fq[:], AF.Sin,
                                 bias=0.0, scale=TWO_PI)

            # ---- k-side: kpre halves -> r0 sins + reductions ----
            Fk = cp.tile([128, NI, L], f16)
            uk = cp.tile([128, R - 1, 2, L], f32)
            ruk = cp.tile([128, R - 1, 2, L], f32)
            fk = cp.tile([128, R - 1, 2, L], f32)
            kpre = [None, None]
            for lh in range(2):
                kp = prepsum.tile([128, QH], f32, tag="qkpre",
                                  name=f"kpre{lh}", bufs=2)
                kpre[lh] = kp
                for db in range(NDB):
                    nc.tensor.matmul(
                        kp[:], W2r[:, db, :],
                        xbT[:, db, lh * QH:(lh + 1) * QH],
                        start=(db == 0), stop=(db == NDB - 1))
                s = slice(lh * QH, (lh + 1) * QH)
                # r0 factors, phi flipped on the k side (0=cos, 1=sin)
                nc.scalar.activation(Fk[:, 0, s], kp[:], AF.Sin,
                                     bias=kb_c[:], scale=OMEGA[0])
                nc.scalar.activation(Fk[:, 1, s], kp[:], AF.Sin,
                                     bias=kb_s[:], scale=OMEGA[0])
                # reductions: slot 1 = sin (u), slot 0 = cos (u + 0.25)
                for j in range(R - 1):
                    nc.vector.tensor_scalar(uk[:, j, 1, s], kp[:],
                                            OMEGA[j + 1] / TWO_PI,
                                            b1u[:, j:j + 1],
                                            op0=ALU.mult, op1=ALU.add)
                nc.vector.tensor_scalar_add(uk[:, :, 0, s], uk[:, :, 1, s],
                                            0.25)
                nc.vector.tensor_scalar(ruk[:, :, :, s], uk[:, :, :, s],
                                        MAGIC, MAGIC,
                                        op0=ALU.add, op1=ALU.subtract)
                nc.vector.tensor_tensor(fk[:, :, :, s], uk[:, :, :, s],
                                        ruk[:, :, :, s], op=ALU.subtract)
                nc.scalar.activation(Fk[:, 2:NI, s], fk[:, :, :, s], AF.Sin,
                                     bias=0.0, scale=TWO_PI)
                if lh == 0:
                    # q-side coefficients while the k second half reduces
                    for i in range(NI):
                        nc.vector.tensor_scalar(
                            Fq[:, i, :], Fq[:, i, :],
                            Acoef[:, i // 2:i // 2 + 1], None, op0=ALU.mult)

            # rounded f32r copy of Xb for the colsum/output matmuls
            Xbr = cp.tile([128, NLB, D], f32r)
            nc.scalar.activation(Xbr[:], Xbs[:], AF.Identity, bias=0.0)

            # ---- colsum(X) doubled + 0.5*b2 broadcast ----
            csp = prepsum.tile([1, D], f32, tag="pre", name="csp")
            for kb in range(NLB):
                nc.tensor.matmul(csp[:], ones_col_r[:], Xbr[:, kb, :],
                                 start=(kb == 0), stop=(kb == NLB - 1))
            csh2 = cp.tile([1, 2, D], f32r)
            nc.vector.tensor_copy(csh2[:, 0, :], csp[:])
            nc.vector.tensor_copy(csh2[:, 1, :], csp[:])
            tpb = prepsum.tile([128, 1], f32, tag="pre", name="tpb")
            nc.tensor.matmul(tpb[:], ones_row[:], b2s[:])
            b2h = cp.tile([128, 1], f32)
            nc.vector.tensor_scalar_mul(b2h[:], tpb[:], 0.5)

            # ---- output accumulators: two banks, two query blocks each;
            # the rank-1 colsum term starts each bank's group ----
            po = [outpsum.tile([128, 2 * D], f32, tag=f"po{h}", name=f"po{h}")
                  for h in range(2)]
            for h in range(2):
                nc.tensor.matmul(po[h][:], ones_row_r[:], csh2[:],
                                 start=True, stop=False, skip_group_check=True)

            # ---- main loop; outputs lag one block so PE never waits ----
            pending = None

            def emit_outs(kb, scT):
                for qs in range(NQB):
                    nc.tensor.matmul(
                        po[qs // 2][:, (qs % 2) * D:(qs % 2 + 1) * D],
                        scT[:, qs * 128:(qs + 1) * 128],
                        Xbr[:, kb, :],
                        start=False,
                        stop=(kb == NLB - 1 and qs % 2 == 1),
                        skip_group_check=True)

            for kb in range(NLB):
                scpre = scorepsum.tile([128, QH], f32, name="scpre")
                for i in range(NI):
                    nc.tensor.matmul(
                        scpre[:], Fk[:, i, kb * 128:(kb + 1) * 128],
                        Fq[:, i, :],
                        start=(i == 0), stop=(i == NI - 1))
                scT = scp.tile([128, QH], f32r, tag="scT", name="scT")
                nc.scalar.activation(scT[:], scpre[:], AF.Tanh,
                                     bias=b2h[:], scale=0.5)
                if pending is not None:
                    emit_outs(*pending)
                pending = (kb, scT)
            emit_outs(*pending)

            # ---- write out: 0.5 * po  (the sigmoid half-factor) ----
            for qs in range(NQB):
                ot = outp.tile([128, D], f32, tag="ot", name="ot")
                nc.vector.tensor_scalar_mul(
                    ot[:], po[qs // 2][:, (qs % 2) * D:(qs % 2 + 1) * D], 0.5)
                eng = nc.sync if qs % 2 == 0 else nc.scalar
                eng.dma_start(out[qs * 128:(qs + 1) * 128, :], ot[:])

    # The act-table chooser picks the first set containing each function,
    # which ping-pongs between exp_and_others (tanh) and trig_and_small
    # (sin).  silu_and_others genuinely contains both sin and tanh;
    # restrict membership (indices unchanged, so emitted set ids stay
    # valid) so one load covers the whole kernel.
    from concourse.hw_specs import get_activation_tables
    tabs = get_activation_tables(nc.m.arch)
    for name, fns in tabs.items():
        if name != "silu_and_others":
            fns.discard(AF.Sin)
            fns.discard(AF.Tanh)

    nc.compile()
    return nc


def _get_nc():
    if "nc" not in _CACHE:
        _CACHE["nc"] = _build_program()
    return _CACHE["nc"]


def kernel(X, W1, W2, W3, bias1, bias2, trace=False):
    global LAST_RESULTS
    from concourse.bass_utils import run_bass_kernel_spmd

    X = np.ascontiguousarray(np.asarray(X, dtype=np.float32))
    W1 = np.ascontiguousarray(np.asarray(W1, dtype=np.float32))
    W2 = np.ascontiguousarray(np.asarray(W2, dtype=np.float32))
    W3 = np.ascontiguousarray(np.asarray(W3, dtype=np.float32))
    b1 = np.ascontiguousarray(np.asarray(bias1, dtype=np.float32).reshape(U, 1))
    b2 = np.ascontiguousarray(np.asarray(bias2, dtype=np.float32).reshape(1, 1))

    nc = _get_nc()
    in_maps = []
    for c in range(N_CORES):
        b, h = c // 2, c % 2
        in_maps.append({
            "Xb": X[b],
            "Xq": np.ascontiguousarray(X[b, h * QH:(h + 1) * QH]),
            "W1": W1,
            "W2": W2,
            "W3v": W3,
            "b1": b1,
            "b2": b2,
        })

    res = run_bass_kernel_spmd(nc, in_maps, core_ids=list(range(N_CORES)),
                               trace=trace)
    LAST_RESULTS = res

    out = np.empty((B, L, D), dtype=np.float32)
    for c in range(N_CORES):
        b, h = c // 2, c % 2
        out[b, h * QH:(h + 1) * QH] = res.results[c]["out"]
    return out


# revision 9
# speedup vs baseline: 6.7844x; 1.0303x over previous
"""Additive attention kernel for Trainium2 (8 NeuronCores, SPMD).

Reference computation (B=4, L=1024, D=256, U=128):
    q = X @ W1                                   [B,L,U]
    k = X @ W2                                   [B,L,U]
    g = tanh(q[:,:,None,:] + k[:,None,:,:] + b1) [B,L,L,U]
    s = sigmoid(g @ W3 + b2)                     [B,L,L]
    out = s @ X                                  [B,L,D]

Sharding: 8 cores = (batch b, query-half h).  Each core handles 512 queries
against all 1024 keys of its batch.

Algorithm: the L*L*U tanh tensor is never materialized.  tanh is
approximated by a 4-term Fourier sine series  tanh(x) ~ sum_r c_r sin(w_r x)
(fit on |x|<=13, gaussian-weighted; end-to-end rel err ~3.5e-3), and
sin(w(q+k)) = sin(wq)cos(wk) + cos(wq)sin(wk) turns the score computation
into a plain matmul with contraction dim 2R*U = 8*128:

    z[q,k] = sum_u W3_u tanh(q_u + k_u + b1_u)
           ~ sum_i Fq_i[u,q] . Fk_i[u,k]        (8 accumulating PE matmuls)

The HW Sin spline is only valid on [-pi, pi]; arguments for the three
higher frequencies are range-reduced with the fp32 magic-constant trick
(f = u - round(u), round via +-1.5*2^23) on DVE; cos comes from a
+0.25-shifted copy of the reduction.  The lowest frequency is in range
directly (cos via +pi/2 bias).

sigmoid(z+b2) = 0.5*tanh(0.5*z + 0.5*b2) + 0.5 keeps everything in one ACT
table set (sin+tanh, silu_and_others).  The +0.5 term becomes a rank-1
colsum(X) matmul accumulated into the output PSUM group and the global 0.5
factor is applied in the final PSUM->SBUF output copies.

All matmuls run in fp16 (1 cycle/row): factors and tanh scores are written
fp16 by ACT directly, X/W get one fp16 cast each (ACT identity for X,
DVE for W).  PE order is software-pipelined (scores of block kb+1 issue
before the outputs of block kb so the PE never waits on tanh), dummy
activations precede the DMA triggers on the scalar queue so the activation
table loads overlap the input DMA, and the gpsimd DMA queue (which sits
behind the ~7us kernel preamble) only carries late-needed tensors.
"""

import numpy as np

B, L, D, U = 4, 1024, 256, 128
QH = L // 2          # queries per core
N_CORES = 8
R = 4

# Fourier fit of tanh on [-13,13], gaussian-weighted (sigma^2=2.67)
OMEGA = [0.2177, 0.7144, 1.48488, 2.49552]
COEF = [1.226339, 0.458469, 0.174421, 0.045835]
MAGIC = float(1.5 * 2 ** 23)
TWO_PI = float(2 * np.pi)

_CACHE = {}
LAST_RESULTS = None


def _build_program():
    import concourse.bass as bass
    import concourse.bacc as bacc
    import concourse.mybir as mybir
    import concourse.tile as tile
    from concourse import masks

    f32 = mybir.dt.float32
    f16 = mybir.dt.float16
    AF = mybir.ActivationFunctionType
    ALU = mybir.AluOpType

    nc = bacc.Bacc(
        "TRN2",
        target_bir_lowering=False,
        debug=False,
        enable_asserts=False,
        num_devices=N_CORES,
    )

    Xb = nc.dram_tensor("Xb", [L, D], f32, kind="ExternalInput")
    Xq = nc.dram_tensor("Xq", [QH, D], f32, kind="ExternalInput")
    W1 = nc.dram_tensor("W1", [D, U], f32, kind="ExternalInput")
    W2 = nc.dram_tensor("W2", [D, U], f32, kind="ExternalInput")
    W3 = nc.dram_tensor("W3v", [U, 1], f32, kind="ExternalInput")
    b1 = nc.dram_tensor("b1", [U, 1], f32, kind="ExternalInput")
    b2 = nc.dram_tensor("b2", [1, 1], f32, kind="ExternalInput")
    out = nc.dram_tensor("out", [QH, D], f32, kind="ExternalOutput")

    NLB = L // 128   # 8 key blocks
    NQB = QH // 128  # 4 query blocks
    NDB = D // 128   # 2 d blocks
    NI = 2 * R       # factor pairs

    with tile.TileContext(nc) as tc:
        with (
            tc.tile_pool(name="const", bufs=1) as cp,
            tc.tile_pool(name="score", bufs=2) as scp,
            tc.tile_pool(name="outs", bufs=2) as outp,
            tc.tile_pool(name="pre_ps", bufs=2, space="PSUM") as prepsum,
            tc.tile_pool(name="score_ps", bufs=2, space="PSUM") as scorepsum,
            tc.tile_pool(name="out_ps", bufs=1, space="PSUM") as outpsum,
        ):
            # ---- constants first: the dummy activations must precede the
            # scalar-queue DMA triggers so the table load runs at t=0 ----
            dum = cp.tile([1, 1], f32)
            nc.vector.memset(dum[:], 0.0)
            dumo = cp.tile([1, 2], f32)
            nc.scalar.activation(dumo[:, 0:1], dum[:], AF.Sin, bias=0.0)
            nc.scalar.activation(dumo[:, 1:2], dum[:], AF.Tanh, bias=0.0)

            # ---- input DMA; queue choice = arrival priority.  sync and
            # scalar start immediately; gpsimd sits behind the preamble ----
            Xqs = cp.tile([128, NQB, D], f32)
            nc.sync.dma_start(
                Xqs[:, 0:NQB // 2, :],
                Xq[0:QH // 2].rearrange("(qb p) d -> p qb d", p=128))
            nc.scalar.dma_start(
                Xqs[:, NQB // 2:, :],
                Xq[QH // 2:QH].rearrange("(qb p) d -> p qb d", p=128))
            W1s = cp.tile([128, NDB, U], f32)
            nc.sync.dma_start(W1s[:], W1[:].rearrange("(db p) u -> p db u", p=128))
            W2s = cp.tile([128, NDB, U], f32)
            nc.scalar.dma_start(W2s[:], W2[:].rearrange("(db p) u -> p db u", p=128))
            Xbs = cp.tile([128, NLB, D], f32)
            nc.sync.dma_start(
                Xbs[:, 0:2, :],
                Xb[0:256].rearrange("(kb p) d -> p kb d", p=128))
            nc.scalar.dma_start(
                Xbs[:, 2:4, :],
                Xb[256:512].rearrange("(kb p) d -> p kb d", p=128))
            b1s = cp.tile([128, 1], f32)
            nc.sync.dma_start(b1s[:], b1[:])
            nc.sync.dma_start(
                Xbs[:, 4:6, :],
                Xb[512:768].rearrange("(kb p) d -> p kb d", p=128))
            nc.gpsimd.dma_start(
                Xbs[:, 6:8, :],
                Xb[768:1024].rearrange("(kb p) d -> p kb d", p=128))
            W3s = cp.tile([128, 1], f32)
            nc.gpsimd.dma_start(W3s[:], W3[:])
            b2s = cp.tile([1, 1], f32)
            nc.gpsimd.dma_start(b2s[:], b2[:])

            ident = cp.tile([128, 128], f32)
            masks.make_identity(nc, ident[:])
            halfpi = cp.tile([128, 1], f32)
            nc.vector.memset(halfpi[:], float(np.pi / 2))
            ones_col = cp.tile([128, 1], f16)
            nc.vector.memset(ones_col[:], 1.0)
            ones_row = cp.tile([1, 128], f16)
            nc.vector.memset(ones_row[:], 1.0)
            ones_row32 = cp.tile([1, 128], f32)
            nc.vector.memset(ones_row32[:], 1.0)

            # fp16 weights (tiny DVE casts)
            W1h = cp.tile([128, NDB, U], f16)
            nc.vector.tensor_copy(W1h[:], W1s[:])
            W2h = cp.tile([128, NDB, U], f16)
            nc.vector.tensor_copy(W2h[:], W2s[:])

            # ---- Xq transposes (PE fp32; DVE copies cast psum -> fp16) ----
            xqT = cp.tile([128, NDB, QH], f16)
            for qb in range(NQB):
                for db in range(NDB):
                    tp = prepsum.tile([128, 128], f32, tag="pre", name="tp")
                    nc.tensor.transpose(
                        tp[:], Xqs[:, qb, db * 128:(db + 1) * 128], ident[:])
                    nc.vector.tensor_copy(
                        xqT[:, db, qb * 128:(qb + 1) * 128], tp[:])

            # ---- qT[u,q] = W1^T Xq^T (stays in PSUM; ACT/DVE read it) ----
            qpre = prepsum.tile([128, QH], f32, tag="qkpre", name="qpre",
                                bufs=2)
            for db in range(NDB):
                nc.tensor.matmul(qpre[:], W1h[:, db, :], xqT[:, db, :],
                                 start=(db == 0), stop=(db == NDB - 1))

            # ---- q-side factors ----
            Fq = cp.tile([128, NI, QH], f16)
            nc.scalar.activation(Fq[:, 0, :], qpre[:], AF.Sin,
                                 bias=0.0, scale=OMEGA[0])
            nc.scalar.activation(Fq[:, 1, :], qpre[:], AF.Sin,
                                 bias=halfpi[:], scale=OMEGA[0])
            # reductions: slot 0 = sin (u), slot 1 = cos (u + 0.25)
            uq = cp.tile([128, R - 1, 2, QH], f32)
            for j in range(R - 1):
                nc.vector.tensor_scalar_mul(uq[:, j, 0, :], qpre[:],
                                            OMEGA[j + 1] / TWO_PI)
            nc.vector.tensor_scalar_add(uq[:, :, 1, :], uq[:, :, 0, :], 0.25)

            # k-side r0 bias tiles (fold b1) + per-frequency b1 offsets
            kb_s = cp.tile([128, 1], f32)
            nc.vector.tensor_scalar_mul(kb_s[:], b1s[:], OMEGA[0])
            kb_c = cp.tile([128, 1], f32)
            nc.vector.tensor_scalar(kb_c[:], b1s[:], OMEGA[0],
                                    float(np.pi / 2),
                                    op0=ALU.mult, op1=ALU.add)
            b1u = cp.tile([128, R - 1], f32)
            for j in range(R - 1):
                nc.vector.tensor_scalar_mul(b1u[:, j:j + 1], b1s[:],
                                            OMEGA[j + 1] / TWO_PI)

            # ---- Xb transposes ----
            xbT = cp.tile([128, NDB, L], f16)
            for kb in range(NLB):
                for db in range(NDB):
                    tp = prepsum.tile([128, 128], f32, tag="pre", name="tp")
                    nc.tensor.transpose(
                        tp[:], Xbs[:, kb, db * 128:(db + 1) * 128], ident[:])
                    nc.vector.tensor_copy(
                        xbT[:, db, kb * 128:(kb + 1) * 128], tp[:])

            # finish q reduction on DVE, then the merged q sine on ACT
            ruq = cp.tile([128, R - 1, 2, QH], f32)
            nc.vector.tensor_scalar(ruq[:], uq[:], MAGIC, MAGIC,
                                    op0=ALU.add, op1=ALU.subtract)
            fq = cp.tile([128, R - 1, 2, QH], f32)
            nc.vector.tensor_tensor(fq[:], uq[:], ruq[:], op=ALU.subtract)
            nc.scalar.activation(Fq[:, 2:NI, :], fq[:], AF.Sin,
                                 bias=0.0, scale=TWO_PI)

            # fp16 X for the colsum/output matmuls, in halves on idle ACT
            Xbh = cp.tile([128, NLB, D], f16)
            nc.scalar.activation(Xbh[:, 0:NLB // 2, :], Xbs[:, 0:NLB // 2, :],
                                 AF.Identity, bias=0.0)

            # ---- k-side: kpre halves -> r0 sins + reductions ----
            Fk = cp.tile([128, NI, L], f16)
            uk = cp.tile([128, R - 1, 2, L], f32)
            ruk = cp.tile([128, R - 1, 2, L], f32)
            fk = cp.tile([128, R - 1, 2, L], f32)
            for lh in range(2):
                kp = prepsum.tile([128, QH], f32, tag="qkpre",
                                  name=f"kpre{lh}", bufs=2)
                for db in range(NDB):
                    nc.tensor.matmul(
                        kp[:], W2h[:, db, :],
                        xbT[:, db, lh * QH:(lh + 1) * QH],
                        start=(db == 0), stop=(db == NDB - 1))
                s = slice(lh * QH, (lh + 1) * QH)
                # r0 factors, phi flipped on the k side (0=cos, 1=sin)
                nc.scalar.activation(Fk[:, 0, s], kp[:], AF.Sin,
                                     bias=kb_c[:], scale=OMEGA[0])
                nc.scalar.activation(Fk[:, 1, s], kp[:], AF.Sin,
                                     bias=kb_s[:], scale=OMEGA[0])
                # reductions: slot 1 = sin (u), slot 0 = cos (u + 0.25)
                for j in range(R - 1):
                    nc.vector.tensor_scalar(uk[:, j, 1, s], kp[:],
                                            OMEGA[j + 1] / TWO_PI,
                                            b1u[:, j:j + 1],
                                            op0=ALU.mult, op1=ALU.add)
                nc.vector.tensor_scalar_add(uk[:, :, 0, s], uk[:, :, 1, s],
                                            0.25)
                nc.vector.tensor_scalar(ruk[:, :, :, s], uk[:, :, :, s],
                                        MAGIC, MAGIC,
                                        op0=ALU.add, op1=ALU.subtract)
                nc.vector.tensor_tensor(fk[:, :, :, s], uk[:, :, :, s],
                                        ruk[:, :, :, s], op=ALU.subtract)
                nc.scalar.activation(Fk[:, 2:NI, s], fk[:, :, :, s], AF.Sin,
                                     bias=0.0, scale=TWO_PI)
                if lh == 0:
                    # second X half cast + q-side coefficients while the
                    # k second half reduces
                    nc.scalar.activation(Xbh[:, NLB // 2:, :],
                                         Xbs[:, NLB // 2:, :],
                                         AF.Identity, bias=0.0)
                    Acoef = cp.tile([128, R], f32)
                    for r in range(R):
                        nc.vector.tensor_scalar_mul(Acoef[:, r:r + 1],
                                                    W3s[:], COEF[r])
                    for i in range(NI):
                        nc.vector.tensor_scalar(
                            Fq[:, i, :], Fq[:, i, :],
                            Acoef[:, i // 2:i // 2 + 1], None, op0=ALU.mult)

            # ---- colsum(X) doubled + 0.5*b2 broadcast ----
            csp = prepsum.tile([1, D], f32, tag="pre", name="csp")
            for kb in range(NLB):
                nc.tensor.matmul(csp[:], ones_col[:], Xbh[:, kb, :],
                                 start=(kb == 0), stop=(kb == NLB - 1))
            csh2 = cp.tile([1, 2, D], f16)
            nc.vector.tensor_copy(csh2[:, 0, :], csp[:])
            nc.vector.tensor_copy(csh2[:, 1, :], csp[:])
            tpb = prepsum.tile([128, 1], f32, tag="pre", name="tpb")
            nc.tensor.matmul(tpb[:], ones_row32[:], b2s[:])
            b2h = cp.tile([128, 1], f32)
            nc.vector.tensor_scalar_mul(b2h[:], tpb[:], 0.5)

            # ---- output accumulators: two banks, two query blocks each;
            # the rank-1 colsum term starts each bank's group ----
            po = [outpsum.tile([128, 2 * D], f32, tag=f"po{h}", name=f"po{h}")
                  for h in range(2)]
            for h in range(2):
                nc.tensor.matmul(po[h][:], ones_row[:], csh2[:],
                                 start=True, stop=False, skip_group_check=True)

            # ---- main loop; outputs lag one block so PE never waits ----
            pending = None

            def emit_outs(kb, scT):
                for qs in range(NQB):
                    nc.tensor.matmul(
                        po[qs // 2][:, (qs % 2) * D:(qs % 2 + 1) * D],
                        scT[:, qs * 128:(qs + 1) * 128],
                        Xbh[:, kb, :],
                        start=False,
                        stop=(kb == NLB - 1 and qs % 2 == 1),
                        skip_group_check=True)

            for kb in range(NLB):
                scpre = scorepsum.tile([128, QH], f32, name="scpre")
                for i in range(NI):
                    nc.tensor.matmul(
                        scpre[:], Fk[:, i, kb * 128:(kb + 1) * 128],
                        Fq[:, i, :],
                        start=(i == 0), stop=(i == NI - 1))
                scT = scp.tile([128, QH], f16, tag="scT", name="scT")
                nc.scalar.activation(scT[:], scpre[:], AF.Tanh,
                                     bias=b2h[:], scale=0.5)
                if pending is not None:
                    emit_outs(*pending)
                pending = (kb, scT)
            emit_outs(*pending)

            # ---- write out: 0.5 * po  (the sigmoid half-factor) ----
            for qs in range(NQB):
                ot = outp.tile([128, D], f32, tag="ot", name="ot")
                nc.vector.tensor_scalar_mul(
                    ot[:], po[qs // 2][:, (qs % 2) * D:(qs % 2 + 1) * D], 0.5)
                eng = nc.sync if qs % 2 == 0 else nc.scalar
                eng.dma_start(out[qs * 128:(qs + 1) * 128, :], ot[:])

    # The act-table chooser picks the first set containing each function,
    # which ping-pongs between exp_and_others (tanh) and trig_and_small
    # (sin).  silu_and_others genuinely contains both sin and tanh;
    # restrict membership (indices unchanged, so emitted set ids stay
    # valid) so one load covers the whole kernel.
    from concourse.hw_specs import get_activation_tables
    tabs = get_activation_tables(nc.m.arch)
    for name, fns in tabs.items():
        if name != "silu_and_others":
            fns.discard(AF.Sin)
            fns.discard(AF.Tanh)

    nc.compile()
    return nc


def _get_nc():
    if "nc" not in _CACHE:
        _CACHE["nc"] = _build_program()
    return _CACHE["nc"]


def kernel(X, W1, W2, W3, bias1, bias2, trace=False):
    global LAST_RESULTS
    from concourse.bass_utils import run_bass_kernel_spmd

    X = np.ascontiguousarray(np.asarray(X, dtype=np.float32))
    W1 = np.ascontiguousarray(np.asarray(W1, dtype=np.float32))
    W2 = np.ascontiguousarray(np.asarray(W2, dtype=np.float32))
    W3 = np.ascontiguousarray(np.asarray(W3, dtype=np.float32))
    b1 = np.ascontiguousarray(np.asarray(bias1, dtype=np.float32).reshape(U, 1))
    b2 = np.ascontiguousarray(np.asarray(bias2, dtype=np.float32).reshape(1, 1))

    nc = _get_nc()
    in_maps = []
    for c in range(N_CORES):
        b, h = c // 2, c % 2
        in_maps.append({
            "Xb": X[b],
            "Xq": np.ascontiguousarray(X[b, h * QH:(h + 1) * QH]),
            "W1": W1,
            "W2": W2,
            "W3v": W3,
            "b1": b1,
            "b2": b2,
        })

    res = run_bass_kernel_spmd(nc, in_maps, core_ids=list(range(N_CORES)),
                               trace=trace)
    LAST_RESULTS = res

    out = np.empty((B, L, D), dtype=np.float32)
    for c in range(N_CORES):
        b, h = c // 2, c % 2
        out[b, h * QH:(h + 1) * QH] = res.results[c]["out"]
    return out
